# revision 9
# baseline (speedup 1.0000x reference)
"""Bass/Trainium2 kernel for nn_BayesianSTDPModule (STDP + LIF recurrence).

Sharding: tensor-parallel over the output-neuron dim O (128 neurons per core,
8 cores, zero collectives). Each core holds its weight shard [128, 2048]
o-major, the full spike matrix in both b-major and i-major fp16 layouts, and
runs the full 32-step recurrence:

  z_in  = s @ w^T + bias                 (PE, fp16 hi/lo 2-pass, fp32 accum)
  v     = vd*v + z_in ; z = (v>=1) ; v*=(1-z)
  tpre_t = d^t + c_t*s  (analytic -> never materialized)
  dw    = Q^T @ s + ep*d^t*colsum_z[o] x 1  - em*d^t* 1 x colsum_s[i]
          where Q = ep*c_t*z - em*S,  S_t = d*S_{t-1} + z_t  (tpost = d^t + S)
  w     = clip(w + dw, -1, 1)

Outputs per core: z shard [1024,128], v^T shard [128,1024], w shard [128,2048];
host concatenates/transposes to full shapes.
"""

import os
import sys
import functools
import numpy as np

sys.path.insert(0, "/opt/trn_rl_repo")

B, I, O = 1024, 2048, 1024
NCORES = 8
OL = O // NCORES            # 128 output neurons per core
SEQ = int(os.environ.get("STDP_SEQ", "32"))
DT_, TAU, TAU_MEM = 1e-3, 0.02, 0.02
EP = float(np.float32(1e-3))
EM = float(np.float32(1e-3))
W_MIN, W_MAX = -1.0, 1.0
DECAY = float(np.float32(np.exp(np.float32(-DT_ / TAU))))      # d (pre & post)
V_DECAY = float(np.float32(np.exp(np.float32(-DT_ / TAU_MEM))))
V_TH = 1.0

KB = B // 128    # 8  b-chunks
KI = I // 128    # 16 i-chunks
NB = B // 512    # 2  b 512-chunks
NI = I // 512    # 4  i 512-chunks

# fp32-recurrence tables for d^t and c_t = sum_{j<t} d^j, matching the
# reference's fp32 decay chains (t index 1..SEQ at position t-1).
_dp = np.float32(1.0)
_c = np.float32(0.0)
DPOW, CSUM = [], []
for _t in range(SEQ):
    _dp = np.float32(_dp * np.float32(DECAY))
    _c = np.float32(_c * np.float32(DECAY) + np.float32(1.0))
    DPOW.append(float(_dp))
    CSUM.append(float(_c))

SINGLE_PASS_FWD = os.environ.get("STDP_FWD1", "0") == "1"


@functools.lru_cache(maxsize=1)
def _build():
    import concourse.bass as bass
    import concourse.mybir as mybir
    import concourse.tile as tile
    from concourse import bacc
    from contextlib import ExitStack

    f32 = mybir.dt.float32
    f16 = mybir.dt.float16
    Alu = mybir.AluOpType
    Act = mybir.ActivationFunctionType

    nc = bacc.Bacc("TRN2", target_bir_lowering=False, debug=False,
                   num_devices=NCORES)

    s16_d = nc.dram_tensor("s16", [B, I], f16, kind="ExternalInput").ap()
    sT16_d = nc.dram_tensor("sT16", [I, B], f16, kind="ExternalInput").ap()
    w0_d = nc.dram_tensor("w0", [OL, I], f32, kind="ExternalInput").ap()
    bias_d = nc.dram_tensor("bias", [OL, 1], f32, kind="ExternalInput").ap()
    cs_d = nc.dram_tensor("cs_row", [1, I], f16, kind="ExternalInput").ap()
    id32_d = nc.dram_tensor("ident32", [128, 128], f32, kind="ExternalInput").ap()
    on16_d = nc.dram_tensor("ones16", [1, 128], f16, kind="ExternalInput").ap()

    z_d = nc.dram_tensor("z_out", [B, OL], f32, kind="ExternalOutput").ap()
    vT_d = nc.dram_tensor("vT_out", [OL, B], f32, kind="ExternalOutput").ap()
    w_d = nc.dram_tensor("w_out", [OL, I], f32, kind="ExternalOutput").ap()

    with tile.TileContext(nc) as tc, ExitStack() as ctx:
        state = ctx.enter_context(tc.tile_pool(name="state", bufs=1))
        psum_zin = ctx.enter_context(
            tc.tile_pool(name="pzin", bufs=1, space="PSUM"))
        psum_big = ctx.enter_context(
            tc.tile_pool(name="pbig", bufs=1, space="PSUM"))
        psum_wt = ctx.enter_context(
            tc.tile_pool(name="pwt", bufs=1, space="PSUM"))

        # ---- persistent SBUF state ----
        s_sb = state.tile([128, KB, I], f16, tag="s_sb")       # b-major spikes
        sT_sb = state.tile([128, KI, B], f16, tag="sT_sb")     # i-major spikes
        w_sb = state.tile([128, I], f32, tag="w_sb")           # o-major weights
        wT_hi = state.tile([128, KI, OL], f16, tag="wT_hi")    # i-major fp16 hi
        wT_lo = state.tile([128, KI, OL], f16, tag="wT_lo")    # i-major fp16 lo
        v_sb = state.tile([128, B], f32, tag="v_sb")           # o-major membrane
        vt_sb = state.tile([128, B], f32, tag="vt_sb")
        S_sb = state.tile([128, KB, OL], f32, tag="S_sb")      # b-major S trace
        z32 = state.tile([128, B], f32, tag="z32")             # o-major spikes
        zb16 = state.tile([128, KB, OL], f16, tag="zb16")      # b-major spikes
        ztmp = state.tile([128, KB, OL], f32, tag="ztmp")
        Q16 = state.tile([128, KB, OL], f16, tag="Q16")
        CS_sb = state.tile([128, I], f32, tag="CS_sb")         # colsum_s bcast
        cs_row = state.tile([1, I], f16, tag="cs_row")
        bias_sb = state.tile([128, 1], f32, tag="bias_sb")
        id32 = state.tile([128, 128], f32, tag="id32")
        on16 = state.tile([1, 128], f16, tag="on16")
        zero_t = state.tile([128, B], f32, tag="zero_t")
        zmask = state.tile([128, B], mybir.dt.uint8, tag="zmask")
        czs_raw = state.tile([128, 1], f32, tag="czs_raw")
        czs_col = state.tile([128, 1], f32, tag="czs_col")
        zf32 = state.tile([128, KB, OL], f32, tag="zf32")

        # ---- load inputs ----
        nc.sync.dma_start(s_sb[:], s16_d.rearrange("(c p) i -> p c i", p=128))
        nc.sync.dma_start(sT_sb[:], sT16_d.rearrange("(c p) b -> p c b", p=128))
        nc.sync.dma_start(w_sb[:], w0_d[:])
        nc.sync.dma_start(bias_sb[:], bias_d[:])
        nc.sync.dma_start(cs_row[:], cs_d[:])
        nc.sync.dma_start(id32[:], id32_d[:])
        nc.sync.dma_start(on16[:], on16_d[:])

        nc.vector.memset(v_sb[:], 0.0)
        nc.vector.memset(S_sb[:], 0.0)
        nc.gpsimd.memset(zero_t[:], 0.0)

        # ---- broadcast colsum_s across partitions via k=1 matmul ----
        for ni in range(NI):
            cs_ps = psum_zin.tile([128, 512], f32, tag="zin")
            nc.tensor.matmul(cs_ps[:], on16[:], cs_row[:, ni * 512:(ni + 1) * 512])
            nc.scalar.copy(CS_sb[:, ni * 512:(ni + 1) * 512], cs_ps[:])

        # ---- helper: transpose w chunk cc + split into fp16 hi/lo ----
        def wT_chunk(wT_ps, pslot, cc):
            nc.tensor.transpose(wT_ps[:, pslot, :],
                                w_sb[:, cc * 128:(cc + 1) * 128], id32[:])
            nc.scalar.copy(wT_hi[:, cc, :], wT_ps[:, pslot, :])
            if not SINGLE_PASS_FWD:
                nc.vector.scalar_tensor_tensor(
                    wT_lo[:, cc, :], wT_hi[:, cc, :], -1.0, wT_ps[:, pslot, :],
                    op0=Alu.mult, op1=Alu.add)

        for half in range(2):
            wT_ps = psum_wt.tile([128, 8, 128], f32, tag="wt")
            for j in range(8):
                wT_chunk(wT_ps, j, half * 8 + j)

        for t in range(1, SEQ + 1):
            dpt = DPOW[t - 1]
            c_t = CSUM[t - 1]

            # ---- forward: z_in^T[o, b] = w @ s^T (+bias via ACT below) ----
            zin_ps = psum_zin.tile([128, B], f32, tag="zin")
            passes = (wT_hi,) if SINGLE_PASS_FWD else (wT_hi, wT_lo)
            for nb in range(NB):
                bs = slice(nb * 512, (nb + 1) * 512)
                first = True
                for k in range(KI):
                    for wt in passes:
                        nc.tensor.matmul(
                            zin_ps[:, bs], wt[:, k, :], sT_sb[:, k, bs],
                            start=first,
                            stop=(k == KI - 1 and wt is passes[-1]))
                        first = False
                # v = vd*v + bias ; v += z_in
                nc.scalar.activation(vt_sb[:, bs], v_sb[:, bs], Act.Identity,
                                     bias=bias_sb[:], scale=V_DECAY)
                nc.vector.tensor_tensor(v_sb[:, bs], vt_sb[:, bs], zin_ps[:, bs],
                                        op=Alu.add)

            # ---- threshold (+ per-partition spike count), reset ----
            # out = (v >= th); accum_out = reduce_{op1=add}(out) per partition
            nc.vector.tensor_scalar(z32[:], v_sb[:], V_TH, None, op0=Alu.is_ge,
                                    op1=Alu.add, accum_out=czs_raw[:])
            nc.gpsimd.tensor_copy(zmask[:], z32[:])
            nc.vector.copy_predicated(v_sb[:], zmask[:], zero_t[:])
            # czs_col = ep * d^t * colsum_z   (o-indexed, per-partition)
            nc.scalar.activation(czs_col[:], czs_raw[:], Act.Copy,
                                 scale=float(EP * dpt))

            # ---- transpose z to b-major (reuses the z_in PSUM slot) ----
            zt_ps = psum_zin.tile([128, KB, 128], f32, tag="zin")
            for c in range(KB):
                nc.tensor.transpose(zt_ps[:, c, :], z32[:, c * 128:(c + 1) * 128],
                                    id32[:])
                nc.scalar.copy(zb16[:, c, :], zt_ps[:, c, :])
                if t == SEQ:
                    nc.scalar.copy(zf32[:, c, :], zt_ps[:, c, :])

            # ---- S, Q (b-major) ----
            nc.gpsimd.tensor_scalar_mul(ztmp[:], zb16[:], float(EP * c_t))
            nc.vector.scalar_tensor_tensor(S_sb[:], S_sb[:], DECAY, zb16[:],
                                           op0=Alu.mult, op1=Alu.add)
            nc.vector.scalar_tensor_tensor(Q16[:], S_sb[:], -EM, ztmp[:],
                                           op0=Alu.mult, op1=Alu.add)

            # ---- dw matmul + w update + clip + re-transpose, per i-chunk ----
            dw_ps = psum_big.tile([128, I], f32, tag="big")
            for ni in range(NI):
                if ni % 2 == 0:
                    wT_ps = psum_wt.tile([128, 8, 128], f32, tag="wt")
                isl = slice(ni * 512, (ni + 1) * 512)
                for k in range(KB):
                    nc.tensor.matmul(dw_ps[:, isl], Q16[:, k, :],
                                     s_sb[:, k, isl],
                                     start=(k == 0), stop=(k == KB - 1))
                # dw += -em*d^t * CS  (colsum_s rank-1)
                nc.vector.scalar_tensor_tensor(
                    dw_ps[:, isl], CS_sb[:, isl], float(-EM * dpt), dw_ps[:, isl],
                    op0=Alu.mult, op1=Alu.add)
                # w = (dw + czs[o]) + w
                nc.vector.scalar_tensor_tensor(
                    w_sb[:, isl], dw_ps[:, isl], czs_col[:], w_sb[:, isl],
                    op0=Alu.add, op1=Alu.add)
                # clip
                nc.gpsimd.tensor_scalar(w_sb[:, isl], w_sb[:, isl], W_MAX, W_MIN,
                                        op0=Alu.min, op1=Alu.max)
                # re-transpose the 4 [128,128] blocks of this chunk, split hi/lo
                for j in range(4):
                    wT_chunk(wT_ps, (ni % 2) * 4 + j, ni * 4 + j)

        # ---- outputs ----
        nc.sync.dma_start(z_d.rearrange("(c p) o -> p c o", p=128), zf32[:])
        nc.sync.dma_start(vT_d[:], v_sb[:])
        nc.sync.dma_start(w_d[:], w_sb[:])

    nc.compile()
    return nc


_last_results = None


def kernel(input_spikes, weight, bias):
    from concourse.bass_utils import run_bass_kernel_spmd

    global _last_results
    nc = _build()

    s = np.asarray(input_spikes, dtype=np.float32)
    w = np.asarray(weight, dtype=np.float32)
    b = np.asarray(bias, dtype=np.float32)

    s16 = s.astype(np.float16)
    sT16 = np.ascontiguousarray(s16.T)
    cs_row = s.sum(axis=0, dtype=np.float32).astype(np.float16)[None, :]
    id32 = np.eye(128, dtype=np.float32)
    on16 = np.ones((1, 128), dtype=np.float16)

    in_maps = []
    for c in range(NCORES):
        sl = slice(c * OL, (c + 1) * OL)
        in_maps.append({
            "s16": s16,
            "sT16": sT16,
            "w0": np.ascontiguousarray(w[sl]),
            "bias": np.ascontiguousarray(b[sl].reshape(OL, 1)),
            "cs_row": cs_row,
            "ident32": id32,
            "ones16": on16,
        })

    res = run_bass_kernel_spmd(
        nc, in_maps, core_ids=list(range(NCORES)),
        trace=os.environ.get("STDP_TRACE", "0") == "1",
    )
    _last_results = res

    z_full = np.concatenate([res.results[c]["z_out"] for c in range(NCORES)],
                            axis=1)
    v_full = np.ascontiguousarray(
        np.concatenate([res.results[c]["vT_out"] for c in range(NCORES)],
                       axis=0).T)
    w_full = np.concatenate([res.results[c]["w_out"] for c in range(NCORES)],
                            axis=0)
    return z_full.astype(np.float32), v_full.astype(np.float32), \
        w_full.astype(np.float32)


# revision 12
# speedup vs baseline: 1.6038x; 1.6038x over previous
"""Bass/Trainium2 kernel for nn_BayesianSTDPModule (STDP + LIF recurrence).

Sharding: tensor-parallel over the output-neuron dim O (128 neurons per core,
8 cores, zero collectives). Each core holds its weight shard [128, 2048]
o-major, the full spike matrix in both b-major and i-major fp16 layouts, and
runs the full 32-step recurrence:

  z_in  = s @ w^T + bias                 (PE, fp16 hi/lo 2-pass, fp32 accum)
  v     = vd*v + z_in ; z = (v>=1) ; v*=(1-z)
  tpre_t = d^t + c_t*s  (analytic -> never materialized)
  dw    = Q^T @ s + ep*d^t*colsum_z[o] x 1  - em*d^t* 1 x colsum_s[i]
          where Q = ep*c_t*z - em*S,  S_t = d*S_{t-1} + z_t  (tpost = d^t + S)
  w     = clip(w + dw, -1, 1)

Outputs per core: z shard [1024,128], v^T shard [128,1024], w shard [128,2048];
host concatenates/transposes to full shapes.
"""

import os
import sys
import functools
import numpy as np

sys.path.insert(0, "/opt/trn_rl_repo")

B, I, O = 1024, 2048, 1024
NCORES = 8
OL = O // NCORES            # 128 output neurons per core
SEQ = int(os.environ.get("STDP_SEQ", "32"))
DT_, TAU, TAU_MEM = 1e-3, 0.02, 0.02
EP = float(np.float32(1e-3))
EM = float(np.float32(1e-3))
W_MIN, W_MAX = -1.0, 1.0
DECAY = float(np.float32(np.exp(np.float32(-DT_ / TAU))))      # d (pre & post)
V_DECAY = float(np.float32(np.exp(np.float32(-DT_ / TAU_MEM))))
V_TH = 1.0

KB = B // 128    # 8  b-chunks
KI = I // 128    # 16 i-chunks
NB = B // 512    # 2  b 512-chunks
NI = I // 512    # 4  i 512-chunks

# fp32-recurrence tables for d^t and c_t = sum_{j<t} d^j, matching the
# reference's fp32 decay chains (t index 1..SEQ at position t-1).
_dp = np.float32(1.0)
_c = np.float32(0.0)
DPOW, CSUM = [], []
for _t in range(SEQ):
    _dp = np.float32(_dp * np.float32(DECAY))
    _c = np.float32(_c * np.float32(DECAY) + np.float32(1.0))
    DPOW.append(float(_dp))
    CSUM.append(float(_c))

SINGLE_PASS_FWD = os.environ.get("STDP_FWD1", "0") == "1"


@functools.lru_cache(maxsize=1)
def _build():
    import concourse.bass as bass
    import concourse.mybir as mybir
    import concourse.tile as tile
    from concourse import bacc
    from contextlib import ExitStack

    f32 = mybir.dt.float32
    f16 = mybir.dt.float16
    Alu = mybir.AluOpType
    Act = mybir.ActivationFunctionType

    nc = bacc.Bacc("TRN2", target_bir_lowering=False, debug=False,
                   num_devices=NCORES)

    s16_d = nc.dram_tensor("s16", [B, I], f16, kind="ExternalInput").ap()
    sT16_d = nc.dram_tensor("sT16", [I, B], f16, kind="ExternalInput").ap()
    w0_d = nc.dram_tensor("w0", [OL, I], f32, kind="ExternalInput").ap()
    bias_d = nc.dram_tensor("bias", [OL, 1], f32, kind="ExternalInput").ap()
    cs_d = nc.dram_tensor("cs_row", [1, I], f16, kind="ExternalInput").ap()
    id32_d = nc.dram_tensor("ident32", [128, 128], f32, kind="ExternalInput").ap()
    on16_d = nc.dram_tensor("ones16", [1, 128], f16, kind="ExternalInput").ap()

    z_d = nc.dram_tensor("z_out", [B, OL], f32, kind="ExternalOutput").ap()
    vT_d = nc.dram_tensor("vT_out", [OL, B], f32, kind="ExternalOutput").ap()
    w_d = nc.dram_tensor("w_out", [OL, I], f32, kind="ExternalOutput").ap()

    with tile.TileContext(nc) as tc, ExitStack() as ctx:
        state = ctx.enter_context(tc.tile_pool(name="state", bufs=1))
        psum_zin = ctx.enter_context(
            tc.tile_pool(name="pzin", bufs=1, space="PSUM"))
        psum_big = ctx.enter_context(
            tc.tile_pool(name="pbig", bufs=1, space="PSUM"))
        psum_wt = ctx.enter_context(
            tc.tile_pool(name="pwt", bufs=1, space="PSUM"))

        # ---- persistent SBUF state ----
        s_sb = state.tile([128, KB, I], f16, tag="s_sb")       # b-major spikes
        sT_sb = state.tile([128, KI, B], f16, tag="sT_sb")     # i-major spikes
        w_sb = state.tile([128, I], f32, tag="w_sb")           # o-major weights
        wT_hi = state.tile([128, KI, OL], f16, tag="wT_hi")    # i-major fp16 hi
        wT_lo = state.tile([128, KI, OL], f16, tag="wT_lo")    # i-major fp16 lo
        v_sb = state.tile([128, B], f32, tag="v_sb")           # o-major membrane
        vt_sb = state.tile([128, B], f32, tag="vt_sb")
        S_sb = state.tile([128, KB, OL], f32, tag="S_sb")      # b-major S trace
        z32 = state.tile([128, B], f32, tag="z32")             # o-major spikes
        zb16 = state.tile([128, KB, OL], f16, tag="zb16")      # b-major spikes
        ztmp = state.tile([128, KB, OL], f32, tag="ztmp")
        Q16 = state.tile([128, KB, OL], f16, tag="Q16")
        CS_sb = state.tile([128, I], f32, tag="CS_sb")         # colsum_s bcast
        cs_row = state.tile([1, I], f16, tag="cs_row")
        bias_sb = state.tile([128, 1], f32, tag="bias_sb")
        id32 = state.tile([128, 128], f32, tag="id32")
        on16 = state.tile([1, 128], f16, tag="on16")
        zero_t = state.tile([128, B], f32, tag="zero_t")
        zmask = state.tile([128, B], mybir.dt.uint8, tag="zmask")
        czs_half = state.tile([128, 2], f32, tag="czs_half")
        czs_raw = state.tile([128, 1], f32, tag="czs_raw")
        czs_col = state.tile([128, 1], f32, tag="czs_col")
        zf32 = state.tile([128, KB, OL], f32, tag="zf32")

        # ---- load inputs ----
        nc.sync.dma_start(s_sb[:], s16_d.rearrange("(c p) i -> p c i", p=128))
        nc.sync.dma_start(sT_sb[:], sT16_d.rearrange("(c p) b -> p c b", p=128))
        nc.sync.dma_start(w_sb[:], w0_d[:])
        nc.sync.dma_start(bias_sb[:], bias_d[:])
        nc.sync.dma_start(cs_row[:], cs_d[:])
        nc.sync.dma_start(id32[:], id32_d[:])
        nc.sync.dma_start(on16[:], on16_d[:])

        nc.vector.memset(v_sb[:], 0.0)
        nc.vector.memset(S_sb[:], 0.0)
        nc.gpsimd.memset(zero_t[:], 0.0)

        # ---- broadcast colsum_s across partitions via k=1 matmul ----
        for ni in range(NI):
            cs_ps = psum_zin.tile([128, 512], f32, tag="zin")
            nc.tensor.matmul(cs_ps[:], on16[:], cs_row[:, ni * 512:(ni + 1) * 512])
            nc.scalar.copy(CS_sb[:, ni * 512:(ni + 1) * 512], cs_ps[:])

        # ---- helper: transpose w chunk cc + split into fp16 hi/lo ----
        def wT_chunk(wT_ps, pslot, cc):
            nc.tensor.transpose(wT_ps[:, pslot, :],
                                w_sb[:, cc * 128:(cc + 1) * 128], id32[:])
            nc.scalar.copy(wT_hi[:, cc, :], wT_ps[:, pslot, :])
            if not SINGLE_PASS_FWD:
                nc.vector.scalar_tensor_tensor(
                    wT_lo[:, cc, :], wT_hi[:, cc, :], -1.0, wT_ps[:, pslot, :],
                    op0=Alu.mult, op1=Alu.add)

        for half in range(2):
            wT_ps = psum_wt.tile([128, 8, 128], f32, tag="wt")
            for j in range(8):
                wT_chunk(wT_ps, j, half * 8 + j)

        for t in range(1, SEQ + 1):
            dpt = DPOW[t - 1]
            c_t = CSUM[t - 1]

            # ---- forward: z_in^T[o, b] = w @ s^T (+bias via ACT below) ----
            # k-outer so each lhsT is loaded once and reused for both
            # b-halves; v/z processed per b-half to shrink PE gaps.
            zin_ps = psum_zin.tile([128, B], f32, tag="zin")
            passes = (wT_hi,) if SINGLE_PASS_FWD else (wT_hi, wT_lo)
            for k in range(KI):
                for wt in passes:
                    for nb in range(NB):
                        bs = slice(nb * 512, (nb + 1) * 512)
                        nc.tensor.matmul(
                            zin_ps[:, bs], wt[:, k, :], sT_sb[:, k, bs],
                            start=(k == 0 and wt is passes[0]),
                            stop=(k == KI - 1 and wt is passes[-1]))
            zt_ps = psum_zin.tile([128, KB, 128], f32, tag="zin")
            for nb in range(NB):
                bs = slice(nb * 512, (nb + 1) * 512)
                # v = vd*v + bias ; v += z_in
                nc.scalar.activation(vt_sb[:, bs], v_sb[:, bs], Act.Identity,
                                     bias=bias_sb[:], scale=V_DECAY)
                nc.vector.tensor_tensor(v_sb[:, bs], vt_sb[:, bs], zin_ps[:, bs],
                                        op=Alu.add)
                # threshold + per-partition spike count for this half
                nc.vector.tensor_scalar(z32[:, bs], v_sb[:, bs], V_TH, None,
                                        op0=Alu.is_ge, op1=Alu.add,
                                        accum_out=czs_half[:, nb:nb + 1])
                nc.scalar.copy(zmask[:, bs], z32[:, bs])
                nc.vector.copy_predicated(v_sb[:, bs], zmask[:, bs],
                                          zero_t[:, bs])
                # transpose this half of z to b-major
                for c in range(nb * 4, nb * 4 + 4):
                    nc.tensor.transpose(zt_ps[:, c, :],
                                        z32[:, c * 128:(c + 1) * 128], id32[:])
                csl = slice(nb * 4, nb * 4 + 4)
                nc.scalar.copy(zb16[:, csl, :], zt_ps[:, csl, :])
                if t == SEQ:
                    nc.scalar.copy(zf32[:, csl, :], zt_ps[:, csl, :])
                # S, ztmp, Q for this half
                nc.scalar.activation(ztmp[:, csl, :], zb16[:, csl, :], Act.Copy,
                                     scale=float(EP * c_t))
                nc.vector.scalar_tensor_tensor(
                    S_sb[:, csl, :], S_sb[:, csl, :], DECAY, zb16[:, csl, :],
                    op0=Alu.mult, op1=Alu.add)
                nc.vector.scalar_tensor_tensor(
                    Q16[:, csl, :], S_sb[:, csl, :], -EM, ztmp[:, csl, :],
                    op0=Alu.mult, op1=Alu.add)

            # czs_col = ep * d^t * (czs_half0 + czs_half1)
            nc.vector.tensor_tensor(czs_raw[:], czs_half[:, 0:1],
                                    czs_half[:, 1:2], op=Alu.add)
            nc.scalar.activation(czs_col[:], czs_raw[:], Act.Copy,
                                 scale=float(EP * dpt))

            # ---- dw matmul (k-outer: 8 weight loads, 4 reuses each) ----
            dw_ps = psum_big.tile([128, I], f32, tag="big")
            for k in range(KB):
                for ni in range(NI):
                    isl = slice(ni * 512, (ni + 1) * 512)
                    nc.tensor.matmul(dw_ps[:, isl], Q16[:, k, :],
                                     s_sb[:, k, isl],
                                     start=(k == 0), stop=(k == KB - 1))
            # ---- w update + clip + re-transpose, per i-chunk ----
            for ni in range(NI):
                if ni % 2 == 0:
                    wT_ps = psum_wt.tile([128, 8, 128], f32, tag="wt")
                isl = slice(ni * 512, (ni + 1) * 512)
                # dw += -em*d^t * CS  (colsum_s rank-1)
                nc.vector.scalar_tensor_tensor(
                    dw_ps[:, isl], CS_sb[:, isl], float(-EM * dpt), dw_ps[:, isl],
                    op0=Alu.mult, op1=Alu.add)
                # w = (dw + czs[o]) + w
                nc.vector.scalar_tensor_tensor(
                    w_sb[:, isl], dw_ps[:, isl], czs_col[:], w_sb[:, isl],
                    op0=Alu.add, op1=Alu.add)
                # clip
                nc.vector.tensor_scalar(w_sb[:, isl], w_sb[:, isl], W_MAX, W_MIN,
                                        op0=Alu.min, op1=Alu.max)
                # re-transpose the 4 [128,128] blocks of this chunk, split hi/lo
                for j in range(4):
                    wT_chunk(wT_ps, (ni % 2) * 4 + j, ni * 4 + j)

        # ---- outputs ----
        nc.sync.dma_start(z_d.rearrange("(c p) o -> p c o", p=128), zf32[:])
        nc.sync.dma_start(vT_d[:], v_sb[:])
        nc.sync.dma_start(w_d[:], w_sb[:])

    nc.compile()
    return nc


_last_results = None


def kernel(input_spikes, weight, bias):
    from concourse.bass_utils import run_bass_kernel_spmd

    global _last_results
    nc = _build()

    s = np.asarray(input_spikes, dtype=np.float32)
    w = np.asarray(weight, dtype=np.float32)
    b = np.asarray(bias, dtype=np.float32)

    s16 = s.astype(np.float16)
    sT16 = np.ascontiguousarray(s16.T)
    cs_row = s.sum(axis=0, dtype=np.float32).astype(np.float16)[None, :]
    id32 = np.eye(128, dtype=np.float32)
    on16 = np.ones((1, 128), dtype=np.float16)

    in_maps = []
    for c in range(NCORES):
        sl = slice(c * OL, (c + 1) * OL)
        in_maps.append({
            "s16": s16,
            "sT16": sT16,
            "w0": np.ascontiguousarray(w[sl]),
            "bias": np.ascontiguousarray(b[sl].reshape(OL, 1)),
            "cs_row": cs_row,
            "ident32": id32,
            "ones16": on16,
        })

    res = run_bass_kernel_spmd(
        nc, in_maps, core_ids=list(range(NCORES)),
        trace=os.environ.get("STDP_TRACE", "0") == "1",
    )
    _last_results = res

    z_full = np.concatenate([res.results[c]["z_out"] for c in range(NCORES)],
                            axis=1)
    v_full = np.ascontiguousarray(
        np.concatenate([res.results[c]["vT_out"] for c in range(NCORES)],
                       axis=0).T)
    w_full = np.concatenate([res.results[c]["w_out"] for c in range(NCORES)],
                            axis=0)
    return z_full.astype(np.float32), v_full.astype(np.float32), \
        w_full.astype(np.float32)


# revision 16
# speedup vs baseline: 1.6527x; 1.0305x over previous
"""Bass/Trainium2 kernel for nn_BayesianSTDPModule (STDP + LIF recurrence).

Sharding: tensor-parallel over the output-neuron dim O (128 neurons per core,
8 cores, zero collectives). Each core holds its weight shard [128, 2048]
o-major, the full spike matrix in both b-major and i-major fp16 layouts, and
runs the full 32-step recurrence:

  z_in  = s @ w^T + bias                 (PE, fp16 hi/lo 2-pass, fp32 accum)
  v     = vd*v + z_in ; z = (v>=1) ; v*=(1-z)
  tpre_t = d^t + c_t*s  (analytic -> never materialized)
  dw    = Q^T @ s + ep*d^t*colsum_z[o] x 1  - em*d^t* 1 x colsum_s[i]
          where Q = ep*c_t*z - em*S,  S_t = d*S_{t-1} + z_t  (tpost = d^t + S)
  w     = clip(w + dw, -1, 1)

Outputs per core: z shard [1024,128], v^T shard [128,1024], w shard [128,2048];
host concatenates/transposes to full shapes.
"""

import os
import sys
import functools
import numpy as np

sys.path.insert(0, "/opt/trn_rl_repo")

B, I, O = 1024, 2048, 1024
NCORES = 8
OL = O // NCORES            # 128 output neurons per core
SEQ = int(os.environ.get("STDP_SEQ", "32"))
DT_, TAU, TAU_MEM = 1e-3, 0.02, 0.02
EP = float(np.float32(1e-3))
EM = float(np.float32(1e-3))
W_MIN, W_MAX = -1.0, 1.0
DECAY = float(np.float32(np.exp(np.float32(-DT_ / TAU))))      # d (pre & post)
V_DECAY = float(np.float32(np.exp(np.float32(-DT_ / TAU_MEM))))
V_TH = 1.0

KB = B // 128    # 8  b-chunks
KI = I // 128    # 16 i-chunks
NB = B // 512    # 2  b 512-chunks
NI = I // 512    # 4  i 512-chunks

# fp32-recurrence tables for d^t and c_t = sum_{j<t} d^j, matching the
# reference's fp32 decay chains (t index 1..SEQ at position t-1).
_dp = np.float32(1.0)
_c = np.float32(0.0)
DPOW, CSUM = [], []
for _t in range(SEQ):
    _dp = np.float32(_dp * np.float32(DECAY))
    _c = np.float32(_c * np.float32(DECAY) + np.float32(1.0))
    DPOW.append(float(_dp))
    CSUM.append(float(_c))

SINGLE_PASS_FWD = os.environ.get("STDP_FWD1", "0") == "1"


@functools.lru_cache(maxsize=1)
def _build():
    import concourse.bass as bass
    import concourse.mybir as mybir
    import concourse.tile as tile
    from concourse import bacc
    from contextlib import ExitStack

    f32 = mybir.dt.float32
    f16 = mybir.dt.float16
    Alu = mybir.AluOpType
    Act = mybir.ActivationFunctionType

    nc = bacc.Bacc("TRN2", target_bir_lowering=False, debug=False,
                   num_devices=NCORES)

    s16_d = nc.dram_tensor("s16", [B, I], f16, kind="ExternalInput").ap()
    sT16_d = nc.dram_tensor("sT16", [I, B], f16, kind="ExternalInput").ap()
    w0_d = nc.dram_tensor("w0", [OL, I], f32, kind="ExternalInput").ap()
    bias_d = nc.dram_tensor("bias", [OL, 1], f32, kind="ExternalInput").ap()
    cs_d = nc.dram_tensor("cs_row", [1, I], f16, kind="ExternalInput").ap()
    id32_d = nc.dram_tensor("ident32", [128, 128], f32, kind="ExternalInput").ap()
    on16_d = nc.dram_tensor("ones16", [1, 128], f16, kind="ExternalInput").ap()

    z_d = nc.dram_tensor("z_out", [B, OL], f32, kind="ExternalOutput").ap()
    vT_d = nc.dram_tensor("vT_out", [OL, B], f32, kind="ExternalOutput").ap()
    w_d = nc.dram_tensor("w_out", [OL, I], f32, kind="ExternalOutput").ap()

    with tile.TileContext(nc) as tc, ExitStack() as ctx:
        state = ctx.enter_context(tc.tile_pool(name="state", bufs=1))
        psum_zin = ctx.enter_context(
            tc.tile_pool(name="pzin", bufs=1, space="PSUM"))
        psum_zt = ctx.enter_context(
            tc.tile_pool(name="pzt", bufs=1, space="PSUM"))
        psum_big = ctx.enter_context(
            tc.tile_pool(name="pbig", bufs=1, space="PSUM"))
        psum_wt = ctx.enter_context(
            tc.tile_pool(name="pwt", bufs=1, space="PSUM"))

        # ---- persistent SBUF state ----
        s_sb = state.tile([128, KB, I], f16, tag="s_sb")       # b-major spikes
        sT_sb = state.tile([128, KI, B], f16, tag="sT_sb")     # i-major spikes
        w_sb = state.tile([128, I], f32, tag="w_sb")           # o-major weights
        wT_hi = state.tile([128, KI, OL], f16, tag="wT_hi")    # i-major fp16 hi
        wT_lo = state.tile([128, KI, OL], f16, tag="wT_lo")    # i-major fp16 lo
        v_sb = state.tile([128, B], f32, tag="v_sb")           # o-major membrane
        vt_sb = state.tile([128, B], f32, tag="vt_sb")
        S_sb = state.tile([128, KB, OL], f32, tag="S_sb")      # b-major S trace
        z16 = state.tile([128, B], f16, tag="z16")             # o-major spikes
        zb16 = state.tile([128, KB, OL], f16, tag="zb16")      # b-major spikes
        ztmp = state.tile([128, KB, OL], f32, tag="ztmp")
        Q16 = state.tile([128, KB, OL], f16, tag="Q16")
        CS_sb = state.tile([128, I], f32, tag="CS_sb")         # colsum_s bcast
        cs_row = state.tile([1, I], f16, tag="cs_row")
        bias_sb = state.tile([128, 1], f32, tag="bias_sb")
        id32 = state.tile([128, 128], f32, tag="id32")
        id16 = state.tile([128, 128], f16, tag="id16")
        on16 = state.tile([1, 128], f16, tag="on16")
        zero_t = state.tile([128, B], f32, tag="zero_t")
        zmask = state.tile([128, B], mybir.dt.uint8, tag="zmask")
        czs_half = state.tile([128, 2], f32, tag="czs_half")
        czs_raw = state.tile([128, 1], f32, tag="czs_raw")
        czs_col = state.tile([128, 1], f32, tag="czs_col")
        zf32 = state.tile([128, KB, OL], f32, tag="zf32")

        # ---- load inputs (small/critical first so initial w transposes
        # overlap the big spike-matrix transfers) ----
        nc.sync.dma_start(id32[:], id32_d[:])
        nc.sync.dma_start(w_sb[:], w0_d[:])
        nc.sync.dma_start(bias_sb[:], bias_d[:])
        nc.sync.dma_start(on16[:], on16_d[:])
        nc.sync.dma_start(cs_row[:], cs_d[:])
        nc.sync.dma_start(sT_sb[:], sT16_d.rearrange("(c p) b -> p c b", p=128))
        nc.sync.dma_start(s_sb[:], s16_d.rearrange("(c p) i -> p c i", p=128))

        nc.scalar.copy(id16[:], id32[:])
        nc.vector.memset(v_sb[:], 0.0)
        nc.vector.memset(S_sb[:], 0.0)
        nc.gpsimd.memset(zero_t[:], 0.0)

        # ---- broadcast colsum_s across partitions via k=1 matmul ----
        for ni in range(NI):
            cs_ps = psum_zin.tile([128, 512], f32, tag="zin")
            nc.tensor.matmul(cs_ps[:], on16[:], cs_row[:, ni * 512:(ni + 1) * 512])
            nc.scalar.copy(CS_sb[:, ni * 512:(ni + 1) * 512], cs_ps[:])

        # ---- helper: transpose w chunk cc + split into fp16 hi/lo ----
        def wT_chunk(wT_ps, pslot, cc):
            nc.tensor.transpose(wT_ps[:, pslot, :],
                                w_sb[:, cc * 128:(cc + 1) * 128], id32[:])
            nc.scalar.copy(wT_hi[:, cc, :], wT_ps[:, pslot, :])
            if not SINGLE_PASS_FWD:
                nc.vector.scalar_tensor_tensor(
                    wT_lo[:, cc, :], wT_hi[:, cc, :], -1.0, wT_ps[:, pslot, :],
                    op0=Alu.mult, op1=Alu.add)

        for quarter in range(4):
            wT_ps = psum_wt.tile([128, 4, 128], f32, tag="wt")
            for j in range(4):
                wT_chunk(wT_ps, j, quarter * 4 + j)

        for t in range(1, SEQ + 1):
            dpt = DPOW[t - 1]
            c_t = CSUM[t - 1]

            # ---- forward: z_in^T[o, b] = w @ s^T (+bias via ACT below) ----
            # nb-outer: b-half 0 finishes halfway through the forward so its
            # v/z/transpose/S/Q chain hides under half 1's matmuls.
            zin_ps = psum_zin.tile([128, B], f32, tag="zin")
            zt_ps = psum_zt.tile([128, KB, 128], f16, tag="zt")
            passes = (wT_hi,) if SINGLE_PASS_FWD else (wT_hi, wT_lo)
            for nb in range(NB):
                bs = slice(nb * 512, (nb + 1) * 512)
                for k in range(KI):
                    for wt in passes:
                        nc.tensor.matmul(
                            zin_ps[:, bs], wt[:, k, :], sT_sb[:, k, bs],
                            start=(k == 0 and wt is passes[0]),
                            stop=(k == KI - 1 and wt is passes[-1]))
                # v = vd*v + bias ; v += z_in
                nc.scalar.activation(vt_sb[:, bs], v_sb[:, bs], Act.Identity,
                                     bias=bias_sb[:], scale=V_DECAY)
                nc.vector.tensor_tensor(v_sb[:, bs], vt_sb[:, bs], zin_ps[:, bs],
                                        op=Alu.add)
                # threshold + per-partition spike count for this half
                nc.vector.tensor_scalar(z16[:, bs], v_sb[:, bs], V_TH, None,
                                        op0=Alu.is_ge, op1=Alu.add,
                                        accum_out=czs_half[:, nb:nb + 1])
                nc.scalar.copy(zmask[:, bs], z16[:, bs])
                nc.vector.copy_predicated(v_sb[:, bs], zmask[:, bs],
                                          zero_t[:, bs])
                # transpose this half of z to b-major
                for c in range(nb * 4, nb * 4 + 4):
                    nc.tensor.transpose(zt_ps[:, c, :],
                                        z16[:, c * 128:(c + 1) * 128], id16[:])
                csl = slice(nb * 4, nb * 4 + 4)
                nc.scalar.copy(zb16[:, csl, :], zt_ps[:, csl, :])
                if t == SEQ:
                    nc.scalar.copy(zf32[:, csl, :], zt_ps[:, csl, :])
                # S, ztmp, Q for this half
                nc.scalar.activation(ztmp[:, csl, :], zb16[:, csl, :], Act.Copy,
                                     scale=float(EP * c_t))
                nc.vector.scalar_tensor_tensor(
                    S_sb[:, csl, :], S_sb[:, csl, :], DECAY, zb16[:, csl, :],
                    op0=Alu.mult, op1=Alu.add)
                nc.vector.scalar_tensor_tensor(
                    Q16[:, csl, :], S_sb[:, csl, :], -EM, ztmp[:, csl, :],
                    op0=Alu.mult, op1=Alu.add)

            # czs_col = ep * d^t * (czs_half0 + czs_half1)
            nc.vector.tensor_tensor(czs_raw[:], czs_half[:, 0:1],
                                    czs_half[:, 1:2], op=Alu.add)
            nc.scalar.activation(czs_col[:], czs_raw[:], Act.Copy,
                                 scale=float(EP * dpt))

            # ---- dw matmul (k-outer: 8 weight loads, 4 reuses each) ----
            dw_ps = psum_big.tile([128, I], f32, tag="big")
            for k in range(KB):
                for ni in range(NI):
                    isl = slice(ni * 512, (ni + 1) * 512)
                    nc.tensor.matmul(dw_ps[:, isl], Q16[:, k, :],
                                     s_sb[:, k, isl],
                                     start=(k == 0), stop=(k == KB - 1))
            # ---- w update + clip + re-transpose, per i-chunk ----
            for ni in range(NI):
                wT_ps = psum_wt.tile([128, 4, 128], f32, tag="wt")
                isl = slice(ni * 512, (ni + 1) * 512)
                # dw += -em*d^t * CS  (colsum_s rank-1)
                nc.vector.scalar_tensor_tensor(
                    dw_ps[:, isl], CS_sb[:, isl], float(-EM * dpt), dw_ps[:, isl],
                    op0=Alu.mult, op1=Alu.add)
                # w = (dw + czs[o]) + w
                nc.vector.scalar_tensor_tensor(
                    w_sb[:, isl], dw_ps[:, isl], czs_col[:], w_sb[:, isl],
                    op0=Alu.add, op1=Alu.add)
                # clip
                nc.vector.tensor_scalar(w_sb[:, isl], w_sb[:, isl], W_MAX, W_MIN,
                                        op0=Alu.min, op1=Alu.max)
                # re-transpose the 4 [128,128] blocks of this chunk, split hi/lo
                for j in range(4):
                    wT_chunk(wT_ps, j, ni * 4 + j)

        # ---- outputs ----
        nc.sync.dma_start(z_d.rearrange("(c p) o -> p c o", p=128), zf32[:])
        nc.sync.dma_start(vT_d[:], v_sb[:])
        nc.sync.dma_start(w_d[:], w_sb[:])

    nc.compile()
    return nc


_last_results = None


def kernel(input_spikes, weight, bias):
    from concourse.bass_utils import run_bass_kernel_spmd

    global _last_results
    nc = _build()

    s = np.asarray(input_spikes, dtype=np.float32)
    w = np.asarray(weight, dtype=np.float32)
    b = np.asarray(bias, dtype=np.float32)

    s16 = s.astype(np.float16)
    sT16 = np.ascontiguousarray(s16.T)
    cs_row = s.sum(axis=0, dtype=np.float32).astype(np.float16)[None, :]
    id32 = np.eye(128, dtype=np.float32)
    on16 = np.ones((1, 128), dtype=np.float16)

    in_maps = []
    for c in range(NCORES):
        sl = slice(c * OL, (c + 1) * OL)
        in_maps.append({
            "s16": s16,
            "sT16": sT16,
            "w0": np.ascontiguousarray(w[sl]),
            "bias": np.ascontiguousarray(b[sl].reshape(OL, 1)),
            "cs_row": cs_row,
            "ident32": id32,
            "ones16": on16,
        })

    res = run_bass_kernel_spmd(
        nc, in_maps, core_ids=list(range(NCORES)),
        trace=os.environ.get("STDP_TRACE", "0") == "1",
    )
    _last_results = res

    z_full = np.concatenate([res.results[c]["z_out"] for c in range(NCORES)],
                            axis=1)
    v_full = np.ascontiguousarray(
        np.concatenate([res.results[c]["vT_out"] for c in range(NCORES)],
                       axis=0).T)
    w_full = np.concatenate([res.results[c]["w_out"] for c in range(NCORES)],
                            axis=0)
    return z_full.astype(np.float32), v_full.astype(np.float32), \
        w_full.astype(np.float32)


# revision 17
# speedup vs baseline: 1.8865x; 1.1414x over previous
"""Bass/Trainium2 kernel for nn_BayesianSTDPModule (STDP + LIF recurrence).

Sharding: tensor-parallel over the output-neuron dim O (128 neurons per core,
8 cores, zero collectives). Each core holds its weight shard [128, 2048]
o-major, the full spike matrix in both b-major and i-major fp16 layouts, and
runs the full 32-step recurrence:

  z_in  = s @ w^T + bias                 (PE, fp16 hi/lo 2-pass, fp32 accum)
  v     = vd*v + z_in ; z = (v>=1) ; v*=(1-z)
  tpre_t = d^t + c_t*s  (analytic -> never materialized)
  dw    = Q^T @ s + ep*d^t*colsum_z[o] x 1  - em*d^t* 1 x colsum_s[i]
          where Q = ep*c_t*z - em*S,  S_t = d*S_{t-1} + z_t  (tpost = d^t + S)
  w     = clip(w + dw, -1, 1)

Outputs per core: z shard [1024,128], v^T shard [128,1024], w shard [128,2048];
host concatenates/transposes to full shapes.
"""

import os
import sys
import functools
import numpy as np

sys.path.insert(0, "/opt/trn_rl_repo")

B, I, O = 1024, 2048, 1024
NCORES = 8
OL = O // NCORES            # 128 output neurons per core
SEQ = int(os.environ.get("STDP_SEQ", "32"))
DT_, TAU, TAU_MEM = 1e-3, 0.02, 0.02
EP = float(np.float32(1e-3))
EM = float(np.float32(1e-3))
W_MIN, W_MAX = -1.0, 1.0
DECAY = float(np.float32(np.exp(np.float32(-DT_ / TAU))))      # d (pre & post)
V_DECAY = float(np.float32(np.exp(np.float32(-DT_ / TAU_MEM))))
V_TH = 1.0

KB = B // 128    # 8  b-chunks
KI = I // 128    # 16 i-chunks
NB = B // 512    # 2  b 512-chunks
NI = I // 512    # 4  i 512-chunks

# fp32-recurrence tables for d^t and c_t = sum_{j<t} d^j, matching the
# reference's fp32 decay chains (t index 1..SEQ at position t-1).
_dp = np.float32(1.0)
_c = np.float32(0.0)
DPOW, CSUM = [], []
for _t in range(SEQ):
    _dp = np.float32(_dp * np.float32(DECAY))
    _c = np.float32(_c * np.float32(DECAY) + np.float32(1.0))
    DPOW.append(float(_dp))
    CSUM.append(float(_c))

SINGLE_PASS_FWD = os.environ.get("STDP_FWD1", "0") == "1"


@functools.lru_cache(maxsize=1)
def _build():
    import concourse.bass as bass
    import concourse.mybir as mybir
    import concourse.tile as tile
    from concourse import bacc
    from contextlib import ExitStack

    f32 = mybir.dt.float32
    f16 = mybir.dt.float16
    Alu = mybir.AluOpType
    Act = mybir.ActivationFunctionType

    nc = bacc.Bacc("TRN2", target_bir_lowering=False, debug=False,
                   num_devices=NCORES)

    s16_d = nc.dram_tensor("s16", [B, I], f16, kind="ExternalInput").ap()
    sT16_d = nc.dram_tensor("sT16", [I, B], f16, kind="ExternalInput").ap()
    w0_d = nc.dram_tensor("w0", [OL, I], f32, kind="ExternalInput").ap()
    bias_d = nc.dram_tensor("bias", [OL, 1], f32, kind="ExternalInput").ap()
    cs_d = nc.dram_tensor("cs_row", [1, I], f16, kind="ExternalInput").ap()
    id32_d = nc.dram_tensor("ident32", [128, 128], f32, kind="ExternalInput").ap()
    on16_d = nc.dram_tensor("ones16", [1, 128], f16, kind="ExternalInput").ap()

    z_d = nc.dram_tensor("z_out", [B, OL], f32, kind="ExternalOutput").ap()
    vT_d = nc.dram_tensor("vT_out", [OL, B], f32, kind="ExternalOutput").ap()
    w_d = nc.dram_tensor("w_out", [OL, I], f32, kind="ExternalOutput").ap()

    with tile.TileContext(nc) as tc, ExitStack() as ctx:
        state = ctx.enter_context(tc.tile_pool(name="state", bufs=1))
        psum_zin = ctx.enter_context(
            tc.tile_pool(name="pzin", bufs=1, space="PSUM"))
        psum_zt = ctx.enter_context(
            tc.tile_pool(name="pzt", bufs=1, space="PSUM"))
        psum_big = ctx.enter_context(
            tc.tile_pool(name="pbig", bufs=1, space="PSUM"))
        psum_wt = ctx.enter_context(
            tc.tile_pool(name="pwt", bufs=1, space="PSUM"))

        # ---- persistent SBUF state ----
        s_sb = state.tile([128, KB, I], f16, tag="s_sb")       # b-major spikes
        sT_sb = state.tile([128, KI, B], f16, tag="sT_sb")     # i-major spikes
        w_sb = state.tile([128, I], f32, tag="w_sb")           # o-major weights
        wT_hi = state.tile([128, KI, OL], f16, tag="wT_hi")    # i-major fp16 hi
        wT_lo = state.tile([128, KI, OL], f16, tag="wT_lo")    # i-major fp16 lo
        v_sb = state.tile([128, B], f32, tag="v_sb")           # o-major membrane
        vt_sb = state.tile([128, B], f32, tag="vt_sb")
        S_sb = state.tile([128, KB, OL], f32, tag="S_sb")      # b-major S trace
        z16 = state.tile([128, B], f16, tag="z16")             # o-major spikes
        zb16 = state.tile([128, KB, OL], f16, tag="zb16")      # b-major spikes
        ztmp = state.tile([128, KB, OL], f32, tag="ztmp")
        Q16 = state.tile([128, KB, OL], f16, tag="Q16")
        CS_sb = state.tile([128, I], f32, tag="CS_sb")         # colsum_s bcast
        cs_row = state.tile([1, I], f16, tag="cs_row")
        bias_sb = state.tile([128, 1], f32, tag="bias_sb")
        id32 = state.tile([128, 128], f32, tag="id32")
        id16 = state.tile([128, 128], f16, tag="id16")
        on16 = state.tile([1, 128], f16, tag="on16")
        zero_t = state.tile([128, B], f32, tag="zero_t")
        zmask = state.tile([128, B], mybir.dt.uint8, tag="zmask")
        czs_half = state.tile([128, 2], f32, tag="czs_half")
        czs_raw = state.tile([128, 1], f32, tag="czs_raw")
        czs_col = state.tile([128, 1], f32, tag="czs_col")
        zf32 = state.tile([128, KB, OL], f32, tag="zf32")

        # ---- load inputs (small/critical first so initial w transposes
        # overlap the big spike-matrix transfers) ----
        nc.sync.dma_start(id32[:], id32_d[:])
        nc.sync.dma_start(w_sb[:], w0_d[:])
        nc.sync.dma_start(bias_sb[:], bias_d[:])
        nc.sync.dma_start(on16[:], on16_d[:])
        nc.sync.dma_start(cs_row[:], cs_d[:])
        nc.sync.dma_start(sT_sb[:], sT16_d.rearrange("(c p) b -> p c b", p=128))
        nc.sync.dma_start(s_sb[:], s16_d.rearrange("(c p) i -> p c i", p=128))

        nc.scalar.copy(id16[:], id32[:])
        nc.vector.memset(v_sb[:], 0.0)
        nc.vector.memset(S_sb[:], 0.0)
        nc.gpsimd.memset(zero_t[:], 0.0)

        # ---- broadcast colsum_s across partitions via k=1 matmul ----
        for ni in range(NI):
            cs_ps = psum_zin.tile([128, 512], f32, tag="zin")
            nc.tensor.matmul(cs_ps[:], on16[:], cs_row[:, ni * 512:(ni + 1) * 512])
            nc.scalar.copy(CS_sb[:, ni * 512:(ni + 1) * 512], cs_ps[:])

        # ---- helper: transpose w chunk cc + split into fp16 hi/lo ----
        def wT_chunk(wT_ps, pslot, cc):
            nc.tensor.transpose(wT_ps[:, pslot, :],
                                w_sb[:, cc * 128:(cc + 1) * 128], id32[:])
            nc.scalar.copy(wT_hi[:, cc, :], wT_ps[:, pslot, :])
            if not SINGLE_PASS_FWD:
                nc.vector.scalar_tensor_tensor(
                    wT_lo[:, cc, :], wT_hi[:, cc, :], -1.0, wT_ps[:, pslot, :],
                    op0=Alu.mult, op1=Alu.add)

        for quarter in range(4):
            wT_ps = psum_wt.tile([128, 4, 128], f32, tag="wt")
            for j in range(4):
                wT_chunk(wT_ps, j, quarter * 4 + j)

        for t in range(1, SEQ + 1):
            dpt = DPOW[t - 1]
            c_t = CSUM[t - 1]

            # ---- forward: z_in^T[o, b] = w @ s^T (+bias via ACT below) ----
            # nb-outer: b-half 0 finishes halfway through the forward so its
            # v/z/transpose/S/Q chain hides under half 1's matmuls.
            zin_ps = psum_zin.tile([128, B], f32, tag="zin")
            zt_ps = psum_zt.tile([128, KB, 128], f16, tag="zt")
            passes = (wT_hi,) if SINGLE_PASS_FWD else (wT_hi, wT_lo)
            for nb in range(NB):
                bs = slice(nb * 512, (nb + 1) * 512)
                for k in range(KI):
                    for wt in passes:
                        nc.tensor.matmul(
                            zin_ps[:, bs], wt[:, k, :], sT_sb[:, k, bs],
                            start=(k == 0 and wt is passes[0]),
                            stop=(k == KI - 1 and wt is passes[-1]))
                # v = vd*v + bias ; v += z_in
                nc.scalar.activation(vt_sb[:, bs], v_sb[:, bs], Act.Identity,
                                     bias=bias_sb[:], scale=V_DECAY)
                nc.vector.tensor_tensor(v_sb[:, bs], vt_sb[:, bs], zin_ps[:, bs],
                                        op=Alu.add)
                # threshold + per-partition spike count for this half
                nc.vector.tensor_scalar(z16[:, bs], v_sb[:, bs], V_TH, None,
                                        op0=Alu.is_ge, op1=Alu.add,
                                        accum_out=czs_half[:, nb:nb + 1])
                nc.scalar.copy(zmask[:, bs], z16[:, bs])
                nc.vector.copy_predicated(v_sb[:, bs], zmask[:, bs],
                                          zero_t[:, bs])
                # transpose this half of z to b-major
                for c in range(nb * 4, nb * 4 + 4):
                    nc.tensor.transpose(zt_ps[:, c, :],
                                        z16[:, c * 128:(c + 1) * 128], id16[:])
                csl = slice(nb * 4, nb * 4 + 4)
                nc.scalar.copy(zb16[:, csl, :], zt_ps[:, csl, :])
                if t == SEQ:
                    nc.scalar.copy(zf32[:, csl, :], zt_ps[:, csl, :])
                # S, ztmp, Q for this half
                nc.scalar.activation(ztmp[:, csl, :], zb16[:, csl, :], Act.Copy,
                                     scale=float(EP * c_t))
                nc.vector.scalar_tensor_tensor(
                    S_sb[:, csl, :], S_sb[:, csl, :], DECAY, zb16[:, csl, :],
                    op0=Alu.mult, op1=Alu.add)
                nc.vector.scalar_tensor_tensor(
                    Q16[:, csl, :], S_sb[:, csl, :], -EM, ztmp[:, csl, :],
                    op0=Alu.mult, op1=Alu.add)

            # czs_col = ep * d^t * (czs_half0 + czs_half1)
            nc.vector.tensor_tensor(czs_raw[:], czs_half[:, 0:1],
                                    czs_half[:, 1:2], op=Alu.add)
            nc.scalar.activation(czs_col[:], czs_raw[:], Act.Copy,
                                 scale=float(EP * dpt))

            # ---- dw matmul: b-half 0 chunks first (Q half 1 may still be
            # in flight), half 1 after; ni slices finish staggered ----
            dw_ps = psum_big.tile([128, I], f32, tag="big")
            for kh in range(2):
                for ni in range(NI):
                    isl = slice(ni * 512, (ni + 1) * 512)
                    for k in range(kh * 4, kh * 4 + 4):
                        nc.tensor.matmul(dw_ps[:, isl], Q16[:, k, :],
                                         s_sb[:, k, isl],
                                         start=(k == 0), stop=(k == KB - 1))
            # ---- w update + clip + re-transpose, per i-chunk ----
            for ni in range(NI):
                wT_ps = psum_wt.tile([128, 4, 128], f32, tag="wt")
                isl = slice(ni * 512, (ni + 1) * 512)
                # dw += -em*d^t * CS  (colsum_s rank-1)
                nc.vector.scalar_tensor_tensor(
                    dw_ps[:, isl], CS_sb[:, isl], float(-EM * dpt), dw_ps[:, isl],
                    op0=Alu.mult, op1=Alu.add)
                # w = (dw + czs[o]) + w
                nc.vector.scalar_tensor_tensor(
                    w_sb[:, isl], dw_ps[:, isl], czs_col[:], w_sb[:, isl],
                    op0=Alu.add, op1=Alu.add)
                # clip
                nc.vector.tensor_scalar(w_sb[:, isl], w_sb[:, isl], W_MAX, W_MIN,
                                        op0=Alu.min, op1=Alu.max)
                # re-transpose the 4 [128,128] blocks of this chunk, split hi/lo
                for j in range(4):
                    wT_chunk(wT_ps, j, ni * 4 + j)

        # ---- outputs ----
        nc.sync.dma_start(z_d.rearrange("(c p) o -> p c o", p=128), zf32[:])
        nc.sync.dma_start(vT_d[:], v_sb[:])
        nc.sync.dma_start(w_d[:], w_sb[:])

    nc.compile()
    return nc


_last_results = None


def kernel(input_spikes, weight, bias):
    from concourse.bass_utils import run_bass_kernel_spmd

    global _last_results
    nc = _build()

    s = np.asarray(input_spikes, dtype=np.float32)
    w = np.asarray(weight, dtype=np.float32)
    b = np.asarray(bias, dtype=np.float32)

    s16 = s.astype(np.float16)
    sT16 = np.ascontiguousarray(s16.T)
    cs_row = s.sum(axis=0, dtype=np.float32).astype(np.float16)[None, :]
    id32 = np.eye(128, dtype=np.float32)
    on16 = np.ones((1, 128), dtype=np.float16)

    in_maps = []
    for c in range(NCORES):
        sl = slice(c * OL, (c + 1) * OL)
        in_maps.append({
            "s16": s16,
            "sT16": sT16,
            "w0": np.ascontiguousarray(w[sl]),
            "bias": np.ascontiguousarray(b[sl].reshape(OL, 1)),
            "cs_row": cs_row,
            "ident32": id32,
            "ones16": on16,
        })

    res = run_bass_kernel_spmd(
        nc, in_maps, core_ids=list(range(NCORES)),
        trace=os.environ.get("STDP_TRACE", "0") == "1",
    )
    _last_results = res

    z_full = np.concatenate([res.results[c]["z_out"] for c in range(NCORES)],
                            axis=1)
    v_full = np.ascontiguousarray(
        np.concatenate([res.results[c]["vT_out"] for c in range(NCORES)],
                       axis=0).T)
    w_full = np.concatenate([res.results[c]["w_out"] for c in range(NCORES)],
                            axis=0)
    return z_full.astype(np.float32), v_full.astype(np.float32), \
        w_full.astype(np.float32)


# revision 18
# speedup vs baseline: 2.4274x; 1.2867x over previous
"""Bass/Trainium2 kernel for nn_BayesianSTDPModule (STDP + LIF recurrence).

Sharding: tensor-parallel over the output-neuron dim O (128 neurons per core,
8 cores, zero collectives). Each core holds its weight shard [128, 2048]
o-major, the full spike matrix in both b-major and i-major fp16 layouts, and
runs the full 32-step recurrence:

  z_in  = s @ w^T + bias                 (PE, fp16 hi/lo 2-pass, fp32 accum)
  v     = vd*v + z_in ; z = (v>=1) ; v*=(1-z)
  tpre_t = d^t + c_t*s  (analytic -> never materialized)
  dw    = Q^T @ s + ep*d^t*colsum_z[o] x 1  - em*d^t* 1 x colsum_s[i]
          where Q = ep*c_t*z - em*S,  S_t = d*S_{t-1} + z_t  (tpost = d^t + S)
  w     = clip(w + dw, -1, 1)

Outputs per core: z shard [1024,128], v^T shard [128,1024], w shard [128,2048];
host concatenates/transposes to full shapes.
"""

import os
import sys
import functools
import numpy as np

sys.path.insert(0, "/opt/trn_rl_repo")

B, I, O = 1024, 2048, 1024
NCORES = 8
OL = O // NCORES            # 128 output neurons per core
SEQ = int(os.environ.get("STDP_SEQ", "32"))
DT_, TAU, TAU_MEM = 1e-3, 0.02, 0.02
EP = float(np.float32(1e-3))
EM = float(np.float32(1e-3))
W_MIN, W_MAX = -1.0, 1.0
DECAY = float(np.float32(np.exp(np.float32(-DT_ / TAU))))      # d (pre & post)
V_DECAY = float(np.float32(np.exp(np.float32(-DT_ / TAU_MEM))))
V_TH = 1.0

KB = B // 128    # 8  b-chunks
KI = I // 128    # 16 i-chunks
NB = B // 512    # 2  b 512-chunks
NI = I // 512    # 4  i 512-chunks

# fp32-recurrence tables for d^t and c_t = sum_{j<t} d^j, matching the
# reference's fp32 decay chains (t index 1..SEQ at position t-1).
_dp = np.float32(1.0)
_c = np.float32(0.0)
DPOW, CSUM = [], []
for _t in range(SEQ):
    _dp = np.float32(_dp * np.float32(DECAY))
    _c = np.float32(_c * np.float32(DECAY) + np.float32(1.0))
    DPOW.append(float(_dp))
    CSUM.append(float(_c))

SINGLE_PASS_FWD = os.environ.get("STDP_FWD1", "0") == "1"


@functools.lru_cache(maxsize=1)
def _build():
    import concourse.bass as bass
    import concourse.mybir as mybir
    import concourse.tile as tile
    from concourse import bacc
    from contextlib import ExitStack

    f32 = mybir.dt.float32
    f16 = mybir.dt.float16
    Alu = mybir.AluOpType
    Act = mybir.ActivationFunctionType

    nc = bacc.Bacc("TRN2", target_bir_lowering=False, debug=False,
                   num_devices=NCORES)

    s16_d = nc.dram_tensor("s16", [B, I], f16, kind="ExternalInput").ap()
    sT16_d = nc.dram_tensor("sT16", [I, B], f16, kind="ExternalInput").ap()
    w0_d = nc.dram_tensor("w0", [OL, I], f32, kind="ExternalInput").ap()
    bias_d = nc.dram_tensor("bias", [OL, 1], f32, kind="ExternalInput").ap()
    cs_d = nc.dram_tensor("cs_row", [1, I], f16, kind="ExternalInput").ap()
    id32_d = nc.dram_tensor("ident32", [128, 128], f32, kind="ExternalInput").ap()
    on16_d = nc.dram_tensor("ones16", [1, 128], f16, kind="ExternalInput").ap()

    z_d = nc.dram_tensor("z_out", [B, OL], f32, kind="ExternalOutput").ap()
    vT_d = nc.dram_tensor("vT_out", [OL, B], f32, kind="ExternalOutput").ap()
    w_d = nc.dram_tensor("w_out", [OL, I], f32, kind="ExternalOutput").ap()

    with tile.TileContext(nc) as tc, ExitStack() as ctx:
        state = ctx.enter_context(tc.tile_pool(name="state", bufs=1))
        psum_zin = ctx.enter_context(
            tc.tile_pool(name="pzin", bufs=1, space="PSUM"))
        psum_zt = ctx.enter_context(
            tc.tile_pool(name="pzt", bufs=1, space="PSUM"))
        psum_big = ctx.enter_context(
            tc.tile_pool(name="pbig", bufs=1, space="PSUM"))
        psum_wt = ctx.enter_context(
            tc.tile_pool(name="pwt", bufs=1, space="PSUM"))

        # ---- persistent SBUF state ----
        s_sb = state.tile([128, KB, I], f16, tag="s_sb")       # b-major spikes
        sT_sb = state.tile([128, KI, B], f16, tag="sT_sb")     # i-major spikes
        w_sb = state.tile([128, I], f32, tag="w_sb")           # o-major weights
        wT_hi = state.tile([128, KI, OL], f16, tag="wT_hi")    # i-major fp16 hi
        wT_lo = state.tile([128, KI, OL], f16, tag="wT_lo")    # i-major fp16 lo
        v_sb = state.tile([128, B], f32, tag="v_sb")           # o-major membrane
        vt_sb = state.tile([128, B], f32, tag="vt_sb")
        S_sb = state.tile([128, KB, OL], f32, tag="S_sb")      # b-major S trace
        z16 = state.tile([128, B], f16, tag="z16")             # o-major spikes
        zb16 = state.tile([128, KB, OL], f16, tag="zb16")      # b-major spikes
        ztmp = state.tile([128, KB, OL], f32, tag="ztmp")
        Q16 = state.tile([128, KB, OL], f16, tag="Q16")
        CS_sb = state.tile([128, I], f32, tag="CS_sb")         # colsum_s bcast
        cs_row = state.tile([1, I], f16, tag="cs_row")
        bias_sb = state.tile([128, 1], f32, tag="bias_sb")
        id32 = state.tile([128, 128], f32, tag="id32")
        id16 = state.tile([128, 128], f16, tag="id16")
        on16 = state.tile([1, 128], f16, tag="on16")
        zero_t = state.tile([128, B], f32, tag="zero_t")
        zmask = state.tile([128, B], mybir.dt.uint8, tag="zmask")
        czs_half = state.tile([128, 2], f32, tag="czs_half")
        czs_raw = state.tile([128, 1], f32, tag="czs_raw")
        czs_col = state.tile([128, 1], f32, tag="czs_col")
        zf32 = state.tile([128, KB, OL], f32, tag="zf32")

        # ---- load inputs (small/critical first so initial w transposes
        # overlap the big spike-matrix transfers) ----
        nc.sync.dma_start(id32[:], id32_d[:])
        nc.sync.dma_start(w_sb[:], w0_d[:])
        nc.sync.dma_start(bias_sb[:], bias_d[:])
        nc.sync.dma_start(on16[:], on16_d[:])
        nc.sync.dma_start(cs_row[:], cs_d[:])
        nc.sync.dma_start(sT_sb[:], sT16_d.rearrange("(c p) b -> p c b", p=128))
        nc.sync.dma_start(s_sb[:], s16_d.rearrange("(c p) i -> p c i", p=128))

        nc.scalar.copy(id16[:], id32[:])
        nc.vector.memset(v_sb[:], 0.0)
        nc.vector.memset(S_sb[:], 0.0)
        nc.gpsimd.memset(zero_t[:], 0.0)

        # ---- broadcast colsum_s across partitions via k=1 matmul ----
        for ni in range(NI):
            cs_ps = psum_zin.tile([128, 512], f32, tag="zin")
            nc.tensor.matmul(cs_ps[:], on16[:], cs_row[:, ni * 512:(ni + 1) * 512])
            nc.scalar.copy(CS_sb[:, ni * 512:(ni + 1) * 512], cs_ps[:])

        # ---- helper: transpose one quarter (4 chunks) of w, then split
        # into fp16 hi (and lo) with ONE batched copy per engine so the
        # next forward isn't gated by 16 serial 367ns ACT copies ----
        def wT_quarter(q):
            wT_ps = psum_wt.tile([128, 4, 128], f32, tag="wt")
            for j in range(4):
                cc = q * 4 + j
                nc.tensor.transpose(wT_ps[:, j, :],
                                    w_sb[:, cc * 128:(cc + 1) * 128], id32[:])
            csl = slice(q * 4, q * 4 + 4)
            nc.scalar.copy(wT_hi[:, csl, :], wT_ps[:, :, :])
            if not SINGLE_PASS_FWD:
                nc.vector.scalar_tensor_tensor(
                    wT_lo[:, csl, :], wT_hi[:, csl, :], -1.0, wT_ps[:, :, :],
                    op0=Alu.mult, op1=Alu.add)

        for quarter in range(4):
            wT_quarter(quarter)

        for t in range(1, SEQ + 1):
            dpt = DPOW[t - 1]
            c_t = CSUM[t - 1]

            # ---- forward: z_in^T[o, b] = w @ s^T (+bias via ACT below) ----
            # nb-outer: b-half 0 finishes halfway through the forward so its
            # v/z/transpose/S/Q chain hides under half 1's matmuls.
            zin_ps = psum_zin.tile([128, B], f32, tag="zin")
            zt_ps = psum_zt.tile([128, KB, 128], f16, tag="zt")
            passes = (wT_hi,) if SINGLE_PASS_FWD else (wT_hi, wT_lo)
            for nb in range(NB):
                bs = slice(nb * 512, (nb + 1) * 512)
                for k in range(KI):
                    for wt in passes:
                        nc.tensor.matmul(
                            zin_ps[:, bs], wt[:, k, :], sT_sb[:, k, bs],
                            start=(k == 0 and wt is passes[0]),
                            stop=(k == KI - 1 and wt is passes[-1]))
                # v = vd*v + bias ; v += z_in
                nc.scalar.activation(vt_sb[:, bs], v_sb[:, bs], Act.Identity,
                                     bias=bias_sb[:], scale=V_DECAY)
                nc.vector.tensor_tensor(v_sb[:, bs], vt_sb[:, bs], zin_ps[:, bs],
                                        op=Alu.add)
                # threshold + per-partition spike count for this half
                nc.vector.tensor_scalar(z16[:, bs], v_sb[:, bs], V_TH, None,
                                        op0=Alu.is_ge, op1=Alu.add,
                                        accum_out=czs_half[:, nb:nb + 1])
                nc.scalar.copy(zmask[:, bs], z16[:, bs])
                nc.vector.copy_predicated(v_sb[:, bs], zmask[:, bs],
                                          zero_t[:, bs])
                # transpose this half of z to b-major
                for c in range(nb * 4, nb * 4 + 4):
                    nc.tensor.transpose(zt_ps[:, c, :],
                                        z16[:, c * 128:(c + 1) * 128], id16[:])
                csl = slice(nb * 4, nb * 4 + 4)
                nc.scalar.copy(zb16[:, csl, :], zt_ps[:, csl, :])
                if t == SEQ:
                    nc.scalar.copy(zf32[:, csl, :], zt_ps[:, csl, :])
                # S, ztmp, Q for this half
                nc.scalar.activation(ztmp[:, csl, :], zb16[:, csl, :], Act.Copy,
                                     scale=float(EP * c_t))
                nc.vector.scalar_tensor_tensor(
                    S_sb[:, csl, :], S_sb[:, csl, :], DECAY, zb16[:, csl, :],
                    op0=Alu.mult, op1=Alu.add)
                nc.vector.scalar_tensor_tensor(
                    Q16[:, csl, :], S_sb[:, csl, :], -EM, ztmp[:, csl, :],
                    op0=Alu.mult, op1=Alu.add)

            # czs_col = ep * d^t * (czs_half0 + czs_half1)
            nc.vector.tensor_tensor(czs_raw[:], czs_half[:, 0:1],
                                    czs_half[:, 1:2], op=Alu.add)
            nc.scalar.activation(czs_col[:], czs_raw[:], Act.Copy,
                                 scale=float(EP * dpt))

            # ---- dw matmul: b-half 0 chunks first (Q half 1 may still be
            # in flight), half 1 after; ni slices finish staggered ----
            dw_ps = psum_big.tile([128, I], f32, tag="big")
            for kh in range(2):
                for ni in range(NI):
                    isl = slice(ni * 512, (ni + 1) * 512)
                    for k in range(kh * 4, kh * 4 + 4):
                        nc.tensor.matmul(dw_ps[:, isl], Q16[:, k, :],
                                         s_sb[:, k, isl],
                                         start=(k == 0), stop=(k == KB - 1))
            # ---- w update + clip + re-transpose, per i-chunk ----
            for ni in range(NI):
                isl = slice(ni * 512, (ni + 1) * 512)
                # dw += -em*d^t * CS  (colsum_s rank-1)
                nc.vector.scalar_tensor_tensor(
                    dw_ps[:, isl], CS_sb[:, isl], float(-EM * dpt), dw_ps[:, isl],
                    op0=Alu.mult, op1=Alu.add)
                # w = (dw + czs[o]) + w
                nc.vector.scalar_tensor_tensor(
                    w_sb[:, isl], dw_ps[:, isl], czs_col[:], w_sb[:, isl],
                    op0=Alu.add, op1=Alu.add)
                # clip
                nc.vector.tensor_scalar(w_sb[:, isl], w_sb[:, isl], W_MAX, W_MIN,
                                        op0=Alu.min, op1=Alu.max)
                # re-transpose the 4 [128,128] blocks of this chunk, split hi/lo
                wT_quarter(ni)

        # ---- outputs ----
        nc.sync.dma_start(z_d.rearrange("(c p) o -> p c o", p=128), zf32[:])
        nc.sync.dma_start(vT_d[:], v_sb[:])
        nc.sync.dma_start(w_d[:], w_sb[:])

    nc.compile()
    return nc


_last_results = None


def kernel(input_spikes, weight, bias):
    from concourse.bass_utils import run_bass_kernel_spmd

    global _last_results
    nc = _build()

    s = np.asarray(input_spikes, dtype=np.float32)
    w = np.asarray(weight, dtype=np.float32)
    b = np.asarray(bias, dtype=np.float32)

    s16 = s.astype(np.float16)
    sT16 = np.ascontiguousarray(s16.T)
    cs_row = s.sum(axis=0, dtype=np.float32).astype(np.float16)[None, :]
    id32 = np.eye(128, dtype=np.float32)
    on16 = np.ones((1, 128), dtype=np.float16)

    in_maps = []
    for c in range(NCORES):
        sl = slice(c * OL, (c + 1) * OL)
        in_maps.append({
            "s16": s16,
            "sT16": sT16,
            "w0": np.ascontiguousarray(w[sl]),
            "bias": np.ascontiguousarray(b[sl].reshape(OL, 1)),
            "cs_row": cs_row,
            "ident32": id32,
            "ones16": on16,
        })

    res = run_bass_kernel_spmd(
        nc, in_maps, core_ids=list(range(NCORES)),
        trace=os.environ.get("STDP_TRACE", "0") == "1",
    )
    _last_results = res

    z_full = np.concatenate([res.results[c]["z_out"] for c in range(NCORES)],
                            axis=1)
    v_full = np.ascontiguousarray(
        np.concatenate([res.results[c]["vT_out"] for c in range(NCORES)],
                       axis=0).T)
    w_full = np.concatenate([res.results[c]["w_out"] for c in range(NCORES)],
                            axis=0)
    return z_full.astype(np.float32), v_full.astype(np.float32), \
        w_full.astype(np.float32)


# revision 20
# speedup vs baseline: 2.9525x; 1.2163x over previous
"""Bass/Trainium2 kernel for nn_BayesianSTDPModule (STDP + LIF recurrence).

Sharding: tensor-parallel over the output-neuron dim O (128 neurons per core,
8 cores, zero collectives). Each core holds its weight shard [128, 2048]
o-major, the full spike matrix in both b-major and i-major fp16 layouts, and
runs the full 32-step recurrence:

  z_in  = s @ w^T + bias                 (PE, fp16 hi/lo 2-pass, fp32 accum)
  v     = vd*v + z_in ; z = (v>=1) ; v*=(1-z)
  tpre_t = d^t + c_t*s  (analytic -> never materialized)
  dw    = Q^T @ s + ep*d^t*colsum_z[o] x 1  - em*d^t* 1 x colsum_s[i]
          where Q = ep*c_t*z - em*S,  S_t = d*S_{t-1} + z_t  (tpost = d^t + S)
  w     = clip(w + dw, -1, 1)

Outputs per core: z shard [1024,128], v^T shard [128,1024], w shard [128,2048];
host concatenates/transposes to full shapes.
"""

import os
import sys
import functools
import numpy as np

sys.path.insert(0, "/opt/trn_rl_repo")

B, I, O = 1024, 2048, 1024
NCORES = 8
OL = O // NCORES            # 128 output neurons per core
SEQ = int(os.environ.get("STDP_SEQ", "32"))
DT_, TAU, TAU_MEM = 1e-3, 0.02, 0.02
EP = float(np.float32(1e-3))
EM = float(np.float32(1e-3))
W_MIN, W_MAX = -1.0, 1.0
DECAY = float(np.float32(np.exp(np.float32(-DT_ / TAU))))      # d (pre & post)
V_DECAY = float(np.float32(np.exp(np.float32(-DT_ / TAU_MEM))))
V_TH = 1.0

KB = B // 128    # 8  b-chunks
KI = I // 128    # 16 i-chunks
NB = B // 512    # 2  b 512-chunks
NI = I // 512    # 4  i 512-chunks

# fp32-recurrence tables for d^t and c_t = sum_{j<t} d^j, matching the
# reference's fp32 decay chains (t index 1..SEQ at position t-1).
_dp = np.float32(1.0)
_c = np.float32(0.0)
DPOW, CSUM = [], []
for _t in range(SEQ):
    _dp = np.float32(_dp * np.float32(DECAY))
    _c = np.float32(_c * np.float32(DECAY) + np.float32(1.0))
    DPOW.append(float(_dp))
    CSUM.append(float(_c))

SINGLE_PASS_FWD = os.environ.get("STDP_FWD1", "0") == "1"


@functools.lru_cache(maxsize=1)
def _build():
    import concourse.bass as bass
    import concourse.mybir as mybir
    import concourse.tile as tile
    from concourse import bacc
    from contextlib import ExitStack

    f32 = mybir.dt.float32
    f16 = mybir.dt.float16
    Alu = mybir.AluOpType
    Act = mybir.ActivationFunctionType

    nc = bacc.Bacc("TRN2", target_bir_lowering=False, debug=False,
                   num_devices=NCORES)

    s16_d = nc.dram_tensor("s16", [B, I], f16, kind="ExternalInput").ap()
    sT16_d = nc.dram_tensor("sT16", [I, B], f16, kind="ExternalInput").ap()
    w0_d = nc.dram_tensor("w0", [OL, I], f32, kind="ExternalInput").ap()
    bias_d = nc.dram_tensor("bias", [OL, 1], f32, kind="ExternalInput").ap()
    cs_d = nc.dram_tensor("cs_row", [1, I], f16, kind="ExternalInput").ap()
    id32_d = nc.dram_tensor("ident32", [128, 128], f32, kind="ExternalInput").ap()
    on16_d = nc.dram_tensor("ones16", [1, 128], f16, kind="ExternalInput").ap()

    z_d = nc.dram_tensor("z_out", [B, OL], f32, kind="ExternalOutput").ap()
    vT_d = nc.dram_tensor("vT_out", [OL, B], f32, kind="ExternalOutput").ap()
    w_d = nc.dram_tensor("w_out", [OL, I], f32, kind="ExternalOutput").ap()

    with tile.TileContext(nc) as tc, ExitStack() as ctx:
        state = ctx.enter_context(tc.tile_pool(name="state", bufs=1))
        psum_zin = ctx.enter_context(
            tc.tile_pool(name="pzin", bufs=1, space="PSUM"))
        psum_dw = ctx.enter_context(
            tc.tile_pool(name="pdw", bufs=1, space="PSUM"))
        psum_wt = ctx.enter_context(
            tc.tile_pool(name="pwt", bufs=1, space="PSUM"))

        # ---- persistent SBUF state ----
        s_sb = state.tile([128, KB, I], f16, tag="s_sb")       # b-major spikes
        sT_sb = state.tile([128, KI, B], f16, tag="sT_sb")     # i-major spikes
        w_sb = state.tile([128, I], f32, tag="w_sb")           # o-major weights
        wT_hi = state.tile([128, KI, OL], f16, tag="wT_hi")    # i-major fp16 hi
        wT_lo = state.tile([128, KI, OL], f16, tag="wT_lo")    # i-major fp16 lo
        v_sb = state.tile([128, B], f32, tag="v_sb")           # o-major membrane
        vt_sb = state.tile([128, B], f32, tag="vt_sb")
        S_sb = state.tile([128, KB, OL], f32, tag="S_sb")      # b-major S trace
        z16 = state.tile([128, B], f16, tag="z16")             # o-major spikes
        zb16 = state.tile([128, KB, OL], f16, tag="zb16")      # b-major spikes
        ztmp = state.tile([128, KB, OL], f32, tag="ztmp")
        Q16 = state.tile([128, KB, OL], f16, tag="Q16")
        CS_sb = state.tile([128, I], f32, tag="CS_sb")         # colsum_s bcast
        cs_row = state.tile([1, I], f16, tag="cs_row")
        bias_sb = state.tile([128, 1], f32, tag="bias_sb")
        id32 = state.tile([128, 128], f32, tag="id32")
        id16 = state.tile([128, 128], f16, tag="id16")
        on16 = state.tile([1, 128], f16, tag="on16")
        zero_t = state.tile([128, B], f32, tag="zero_t")
        zmask = state.tile([128, B], mybir.dt.uint8, tag="zmask")
        czs_half = state.tile([128, 2], f32, tag="czs_half")
        czs_raw = state.tile([128, 1], f32, tag="czs_raw")
        czs_col = state.tile([128, 1], f32, tag="czs_col")
        zf32 = state.tile([128, KB, OL], f32, tag="zf32")

        # ---- load inputs (small/critical first so initial w transposes
        # overlap the big spike-matrix transfers) ----
        nc.sync.dma_start(id32[:], id32_d[:])
        nc.sync.dma_start(w_sb[:], w0_d[:])
        nc.sync.dma_start(bias_sb[:], bias_d[:])
        nc.sync.dma_start(on16[:], on16_d[:])
        nc.sync.dma_start(cs_row[:], cs_d[:])
        nc.sync.dma_start(sT_sb[:], sT16_d.rearrange("(c p) b -> p c b", p=128))
        nc.sync.dma_start(s_sb[:], s16_d.rearrange("(c p) i -> p c i", p=128))

        nc.scalar.copy(id16[:], id32[:])
        nc.vector.memset(v_sb[:], 0.0)
        nc.vector.memset(S_sb[:], 0.0)
        nc.gpsimd.memset(zero_t[:], 0.0)

        # ---- broadcast colsum_s across partitions via k=1 matmul ----
        for ni in range(NI):
            cs_ps = psum_zin.tile([128, 512], f32, tag="zin")
            nc.tensor.matmul(cs_ps[:], on16[:], cs_row[:, ni * 512:(ni + 1) * 512])
            nc.scalar.copy(CS_sb[:, ni * 512:(ni + 1) * 512], cs_ps[:])

        # ---- helper: transpose one quarter (4 chunks) of w, then split
        # into fp16 hi (and lo) with ONE batched copy per engine so the
        # next forward isn't gated by 16 serial 367ns ACT copies ----
        def wT_quarter(q):
            wT_ps = psum_wt.tile([128, 4, 128], f32, tag="wt")
            for j in range(4):
                cc = q * 4 + j
                nc.tensor.transpose(wT_ps[:, j, :],
                                    w_sb[:, cc * 128:(cc + 1) * 128], id32[:])
            csl = slice(q * 4, q * 4 + 4)
            nc.scalar.copy(wT_hi[:, csl, :], wT_ps[:, :, :])
            if not SINGLE_PASS_FWD:
                nc.vector.scalar_tensor_tensor(
                    wT_lo[:, csl, :], wT_hi[:, csl, :], -1.0, wT_ps[:, :, :],
                    op0=Alu.mult, op1=Alu.add)

        for quarter in range(4):
            wT_quarter(quarter)

        for t in range(1, SEQ + 1):
            dpt = DPOW[t - 1]
            c_t = CSUM[t - 1]

            # ---- forward: z_in^T[o, b] = w @ s^T (+bias via ACT below) ----
            # nb-outer: b-half 0 finishes halfway through the forward so its
            # v/z/transpose/S/Q chain hides under half 1's matmuls.
            zin_ps = psum_zin.tile([128, B], f32, tag="zin")
            # z-transpose psum shares the dw slice-3 slot (disjoint lifetimes)
            zt_ps = psum_dw.tile([128, KB, 128], f16, tag="dw3")
            passes = (wT_hi,) if SINGLE_PASS_FWD else (wT_hi, wT_lo)
            for nb in range(NB):
                bs = slice(nb * 512, (nb + 1) * 512)
                for k in range(KI):
                    for wt in passes:
                        nc.tensor.matmul(
                            zin_ps[:, bs], wt[:, k, :], sT_sb[:, k, bs],
                            start=(k == 0 and wt is passes[0]),
                            stop=(k == KI - 1 and wt is passes[-1]))
                # v = vd*v + bias ; v += z_in
                nc.scalar.activation(vt_sb[:, bs], v_sb[:, bs], Act.Identity,
                                     bias=bias_sb[:], scale=V_DECAY)
                nc.vector.tensor_tensor(v_sb[:, bs], vt_sb[:, bs], zin_ps[:, bs],
                                        op=Alu.add)
                # threshold + per-partition spike count for this half
                nc.vector.tensor_scalar(z16[:, bs], v_sb[:, bs], V_TH, None,
                                        op0=Alu.is_ge, op1=Alu.add,
                                        accum_out=czs_half[:, nb:nb + 1])
                nc.scalar.copy(zmask[:, bs], z16[:, bs])
                nc.vector.copy_predicated(v_sb[:, bs], zmask[:, bs],
                                          zero_t[:, bs])
                # transpose this half of z to b-major
                for c in range(nb * 4, nb * 4 + 4):
                    nc.tensor.transpose(zt_ps[:, c, :],
                                        z16[:, c * 128:(c + 1) * 128], id16[:])
                csl = slice(nb * 4, nb * 4 + 4)
                nc.scalar.copy(zb16[:, csl, :], zt_ps[:, csl, :])
                if t == SEQ:
                    nc.scalar.copy(zf32[:, csl, :], zt_ps[:, csl, :])
                # S, ztmp, Q for this half
                nc.scalar.activation(ztmp[:, csl, :], zb16[:, csl, :], Act.Copy,
                                     scale=float(EP * c_t))
                nc.vector.scalar_tensor_tensor(
                    S_sb[:, csl, :], S_sb[:, csl, :], DECAY, zb16[:, csl, :],
                    op0=Alu.mult, op1=Alu.add)
                nc.vector.scalar_tensor_tensor(
                    Q16[:, csl, :], S_sb[:, csl, :], -EM, ztmp[:, csl, :],
                    op0=Alu.mult, op1=Alu.add)

            # czs_col = ep * d^t * (czs_half0 + czs_half1)
            nc.vector.tensor_tensor(czs_raw[:], czs_half[:, 0:1],
                                    czs_half[:, 1:2], op=Alu.add)
            nc.scalar.activation(czs_col[:], czs_raw[:], Act.Copy,
                                 scale=float(EP * dpt))

            # ---- dw matmul: b-half 0 chunks first (Q half 1 may still be
            # in flight), half 1 after; one PSUM tile per ni slice so the
            # w-update of slice ni doesn't wait for other slices' matmuls ----
            dw_ps = [psum_dw.tile([128, 512], f32, tag="dw%d" % ni,
                                  name="dw_ps%d_t%d" % (ni, t))
                     for ni in range(NI)]
            for kh in range(2):
                for ni in range(NI):
                    for k in range(kh * 4, kh * 4 + 4):
                        nc.tensor.matmul(dw_ps[ni][:], Q16[:, k, :],
                                         s_sb[:, k, ni * 512:(ni + 1) * 512],
                                         start=(k == 0), stop=(k == KB - 1))
            # ---- w update + clip + re-transpose, per i-chunk ----
            for ni in range(NI):
                isl = slice(ni * 512, (ni + 1) * 512)
                # dw += -em*d^t * CS  (colsum_s rank-1)
                nc.vector.scalar_tensor_tensor(
                    dw_ps[ni][:], CS_sb[:, isl], float(-EM * dpt), dw_ps[ni][:],
                    op0=Alu.mult, op1=Alu.add)
                # w = (dw + czs[o]) + w
                nc.vector.scalar_tensor_tensor(
                    w_sb[:, isl], dw_ps[ni][:], czs_col[:], w_sb[:, isl],
                    op0=Alu.add, op1=Alu.add)
                # clip
                nc.vector.tensor_scalar(w_sb[:, isl], w_sb[:, isl], W_MAX, W_MIN,
                                        op0=Alu.min, op1=Alu.max)
                # re-transpose the 4 [128,128] blocks of this chunk, split hi/lo
                wT_quarter(ni)

        # ---- outputs ----
        nc.sync.dma_start(z_d.rearrange("(c p) o -> p c o", p=128), zf32[:])
        nc.sync.dma_start(vT_d[:], v_sb[:])
        nc.sync.dma_start(w_d[:], w_sb[:])

    nc.compile()
    return nc


_last_results = None


def kernel(input_spikes, weight, bias):
    from concourse.bass_utils import run_bass_kernel_spmd

    global _last_results
    nc = _build()

    s = np.asarray(input_spikes, dtype=np.float32)
    w = np.asarray(weight, dtype=np.float32)
    b = np.asarray(bias, dtype=np.float32)

    s16 = s.astype(np.float16)
    sT16 = np.ascontiguousarray(s16.T)
    cs_row = s.sum(axis=0, dtype=np.float32).astype(np.float16)[None, :]
    id32 = np.eye(128, dtype=np.float32)
    on16 = np.ones((1, 128), dtype=np.float16)

    in_maps = []
    for c in range(NCORES):
        sl = slice(c * OL, (c + 1) * OL)
        in_maps.append({
            "s16": s16,
            "sT16": sT16,
            "w0": np.ascontiguousarray(w[sl]),
            "bias": np.ascontiguousarray(b[sl].reshape(OL, 1)),
            "cs_row": cs_row,
            "ident32": id32,
            "ones16": on16,
        })

    res = run_bass_kernel_spmd(
        nc, in_maps, core_ids=list(range(NCORES)),
        trace=os.environ.get("STDP_TRACE", "0") == "1",
    )
    _last_results = res

    z_full = np.concatenate([res.results[c]["z_out"] for c in range(NCORES)],
                            axis=1)
    v_full = np.ascontiguousarray(
        np.concatenate([res.results[c]["vT_out"] for c in range(NCORES)],
                       axis=0).T)
    w_full = np.concatenate([res.results[c]["w_out"] for c in range(NCORES)],
                            axis=0)
    return z_full.astype(np.float32), v_full.astype(np.float32), \
        w_full.astype(np.float32)
